# revision 9
# baseline (speedup 1.0000x reference)
"""Trainium2 Bass kernel for Luong-style attention.

Reference computation (per full problem):
    h = decoder_hidden @ W.T + b          # [B, De]
    enc = encoder_output.transpose(1,0,2) # [B, S, De]
    a = softmax(einsum('bsd,bd->bs', enc, h), axis=1)
    context = einsum('bs,bsd->bd', a, enc)  # [B, De]

Shapes: B=64, S=4096, Dd=1024, De=512 (f32).

Strategy: data-parallel over B across 8 NeuronCores (B_local=8 each).
encoder_output is the huge tensor (512 MB); each core streams its
64 MB shard from HBM exactly once (chunked two-level softmax).
Per 128-row s-tile (f32, no bf16 copy of the stream):
  - scores via DVE scalar_tensor_tensor (product + row-sum in one op)
    against a partition-broadcast fp16 copy of h (one 2-byte source
    keeps the DVE at full rate; two f32 sources would halve it),
  - per-chunk softmax via PE transpose + ACT exp (bias=-chunk_max,
    fused row-sum),
  - context accumulated in a single [8, 512] PSUM bank: for each b,
    matmul(lhsT=prob column [128,1] fp32r, rhs=enc f32r [128,512]) ->
    out row [1,512] at partition b.  fp32r moving data runs at
    1 cycle/row for N>=256, and a 1-column weight load is ~free, so
    the PE streams the f32 tile directly (no bf16 cast needed).
    The accumulation group is opened by one bank-wide zero matmul
    (start=True clears has_written bits bank-wide).
  - chunks combined at the end with exp(m_c - M)/l weights; partials
    are already in natural [b, d] layout so the combine is a short
    chain of [8,512] DVE ops.
No collectives needed.  Bottleneck is the HBM stream (~187 us for
64 MB/core at ~358 GB/s); all engines sit below that.
"""

import numpy as np

import concourse.bass as bass
import concourse.bacc as bacc_mod
import concourse.tile as tile
import concourse.mybir as mybir
from concourse import masks
from concourse.bass_utils import run_bass_kernel_spmd

F32 = mybir.dt.float32
F32R = mybir.dt.float32r
F16 = mybir.dt.float16
BF16 = mybir.dt.bfloat16
ALU = mybir.AluOpType
ACTF = mybir.ActivationFunctionType
AX = mybir.AxisListType

NCORES = 8
B = 8          # per-core batch
S = 4096
DD = 1024
DE = 512
P = 128        # s-values per tile
NTILES = S // P          # 32
CHUNK_TILES = 4          # s-tiles per softmax chunk
NCHUNK = NTILES // CHUNK_TILES   # 8


def build_nc(ntiles: int = NTILES):
    nchunk = ntiles // CHUNK_TILES
    s_local = ntiles * P
    nc = bacc_mod.Bacc("TRN2", target_bir_lowering=False, debug=False)
    dec_d = nc.dram_tensor("decoder_hidden", [B, DD], F32, kind="ExternalInput")
    enc_d = nc.dram_tensor("encoder_output", [s_local, B, DE], F32R, kind="ExternalInput")
    w_d = nc.dram_tensor("W", [DE, DD], F32, kind="ExternalInput")
    b_d = nc.dram_tensor("b", [DE], F32, kind="ExternalInput")
    out_d = nc.dram_tensor("out", [B, DE], F32, kind="ExternalOutput")

    with tile.TileContext(nc) as tc:
        with (
            tc.tile_pool(name="const", bufs=1) as const_pool,
            tc.tile_pool(name="persist", bufs=1) as persist_pool,
            tc.tile_pool(name="enc", bufs=8) as enc_pool,
            tc.tile_pool(name="junk", bufs=4) as junk_pool,
            tc.tile_pool(name="scores", bufs=6) as sc_pool,
            tc.tile_pool(name="probs", bufs=2) as p_pool,
        ):
            wload_cm = tc.tile_pool(name="wload", bufs=2)
            wload_pool = wload_cm.__enter__()
            wt_cm = tc.tile_pool(name="wt", bufs=1)
            wt_pool = wt_cm.__enter__()
            setup_psum_cm = tc.tile_pool(name="psum_setup", bufs=4, space="PSUM")
            psum_setup = setup_psum_cm.__enter__()
            setup_psum2_cm = tc.tile_pool(name="psum_setup2", bufs=4, space="PSUM")
            psum_setup2 = setup_psum2_cm.__enter__()
            # ---- constants ----
            ident = const_pool.tile([P, P], F32)
            masks.make_identity(nc, ident[:])
            ones = const_pool.tile([1, P], F32)
            nc.vector.memset(ones[:], 1.0)
            zeros_row = const_pool.tile([1, DE], BF16)
            nc.vector.memset(zeros_row[:], 0.0)
            ones_bf = const_pool.tile([1, B], BF16)
            nc.vector.memset(ones_bf[:], 1.0)
            # row-broadcast selectors: sel[:, bb, :] is [8, 128] with row bb
            # all-ones; matmul(sel_bb, x) broadcasts x's row bb to all
            # 128 partitions without any cross-partition DMA.
            sel = const_pool.tile([B, B, P], F32)
            nc.gpsimd.memset(sel[:], 0.0)
            # sel[k, bb, m] = 1.0 iff k == bb  (k*1 + bb*(-1) == 0)
            nc.gpsimd.affine_select(
                out=sel[:], in_=sel[:],
                compare_op=ALU.not_equal, fill=1.0, base=0,
                pattern=[[-1, B], [0, P]], channel_multiplier=1)

            # ---- load small inputs ----
            dec_sb = const_pool.tile([B, DD], F32)
            nc.sync.dma_start(dec_sb[:], dec_d[:])
            bias_sb = const_pool.tile([1, DE], F32)
            nc.sync.dma_start(bias_sb[:], b_d[None, :])
            setup_dmas = []

            # ---- transpose dec: [8,1024] -> decT [128, 8, 8] (chunk c = cols c*128..) ----
            decT = const_pool.tile([P, DD // P, B], F32)
            for c in range(DD // P):
                tp = psum_setup.tile([P, B], F32, tag="su")
                nc.tensor.transpose(tp[:], dec_sb[:, c * P:(c + 1) * P], ident[0:B, 0:B])
                nc.vector.tensor_copy(decT[:, c, :], tp[:])

            # ---- transpose W: [512,1024] -> WT [128, 8, 512] (chunk c = W.T rows c*128..) ----
            wt_sb = wt_pool.tile([P, DD // P, DE], F32)
            for wi in range(DE // P):
                w_row = wload_pool.tile([P, DD], F32, tag="wrow")
                half = DD // 2
                setup_dmas.append(nc.sync.dma_start(
                    w_row[:, 0:half], w_d[wi * P:(wi + 1) * P, 0:half]))
                setup_dmas.append(nc.sync.dma_start(
                    w_row[:, half:DD], w_d[wi * P:(wi + 1) * P, half:DD]))
                for c in range(DD // P):
                    tp = psum_setup.tile([P, P], F32, tag="su")
                    nc.tensor.transpose(tp[:], w_row[:, c * P:(c + 1) * P], ident[:])
                    nc.vector.tensor_copy(wt_sb[:, c, wi * P:(wi + 1) * P], tp[:])

            # ---- h = dec @ W.T + b  -> h_sb [8, 512] ----
            h_ps = psum_setup2.tile([B, DE], F32, tag="hsu")
            for c in range(DD // P):
                nc.tensor.matmul(h_ps[:], decT[:, c, :], wt_sb[:, c, :],
                                 start=(c == 0), stop=False)
            nc.tensor.matmul(h_ps[:], ones[0:1, 0:B], bias_sb[:],
                             start=False, stop=True)
            h_sb = const_pool.tile([B, DE], F32)
            nc.vector.tensor_copy(h_sb[:], h_ps[:])

            # ---- broadcast h along partitions: hb [128, 8, 512] fp16 ----
            # selector matmul: out = sel_bb.T @ h_sb puts h row bb on all
            # 128 partitions; the psum->sbuf copy converts to fp16 so the
            # score stt has only one 4-byte source (full DVE rate).
            hb = persist_pool.tile([P, B, DE], F16)
            for bb in range(B):
                hp = psum_setup2.tile([P, DE], F32, tag="hsu")
                nc.tensor.matmul(hp[:], sel[:, bb, :], h_sb[:],
                                 start=True, stop=True)
                nc.scalar.copy(hb[:, bb, :], hp[:])

            setup_psum2_cm.__exit__(None, None, None)
            setup_psum_cm.__exit__(None, None, None)
            wt_cm.__exit__(None, None, None)
            wload_cm.__exit__(None, None, None)
            _tr_cm = tc.tile_pool(name="psum_tr", bufs=2, space="PSUM")
            psum_tr = _tr_cm.__enter__()
            _sc_cm = tc.tile_pool(name="psum_sc", bufs=2, space="PSUM")
            psum_sc = _sc_cm.__enter__()
            _ctx_cm = tc.tile_pool(name="psum_ctx", bufs=2, space="PSUM")
            psum_ctx = _ctx_cm.__enter__()

            # ---- diagonal prob-weight tiles ----
            # pz[p, b, b'] = p_col_b[p] iff b == b' else 0.  Used as the
            # [128, 8] lhsT for the context matmul of batch b: the single
            # nonzero column routes the output to psum partition b while
            # keeping the matmul's base partition at 0 (PE requires 0/32/64).
            # Off-diagonals are zeroed once here and never written again;
            # each tile only refreshes the 8 diagonal slots (stride-9 AP).
            NPZ = 4
            pzs = [persist_pool.tile([P, B, B], F32R, name=f"pz{i}")
                   for i in range(NPZ)]
            zeros_pz = const_pool.tile([P, B * B], F32)
            nc.vector.memset(zeros_pz[:], 0.0)
            pz_diags = []
            for pz in pzs:
                nc.scalar.copy(pz[:].rearrange("p a b -> p (a b)"), zeros_pz[:])
                pz_diags.append(pz[:].rearrange("p a b -> p (a b)")[:, 0:B * B:B + 1])

            # ---- per-chunk stats / outputs ----
            m_all = persist_pool.tile([B, nchunk], F32)
            negm_all = persist_pool.tile([B, nchunk], F32)
            l_all = persist_pool.tile([B, nchunk], F32)
            w_all = persist_pool.tile([B, nchunk], F32)
            # context partials per chunk, natural [b, d] layout
            ctxc_all = persist_pool.tile([B, nchunk, DE], F32)

            # ---- main streaming loop over S ----
            for c in range(nchunk):
                # open the ctx accumulation bank early (no deps beyond pool
                # rotation) so the PE's in-order queue never stalls on it.
                ctx_ps = psum_ctx.tile([B, DE], F32)
                nc.tensor.matmul(ctx_ps[:], ones_bf[:], zeros_row[:],
                                 start=True, stop=False)
                enc_tiles = []
                scT = psum_sc.tile([B, CHUNK_TILES * P], F32)
                for t in range(CHUNK_TILES):
                    j = c * CHUNK_TILES + t
                    et = enc_pool.tile([P, B, DE], F32R)
                    enc_dma = nc.sync.dma_start(et[:], enc_d[j * P:(j + 1) * P, :, :])
                    if j == 0:
                        for sd in setup_dmas:
                            tile.add_dep_helper(enc_dma.ins, sd.ins,
                                                reason="let setup W loads win HBM first")
                    enc_tiles.append(et)
                    # scores for this tile: [128, 8]
                    sct = sc_pool.tile([P, B], F32)
                    for bb in range(B):
                        junk = junk_pool.tile([P, DE], BF16, tag="junk")
                        nc.vector.scalar_tensor_tensor(
                            out=junk[:],
                            in0=et[:, bb, :].bitcast(F32),
                            scalar=1.0,
                            in1=hb[:, bb, :],
                            op0=ALU.mult,
                            op1=ALU.mult,
                            accum_out=sct[:, bb:bb + 1],
                        )
                    # transpose scores into [8, 128] slice of chunk psum
                    nc.tensor.transpose(scT[:, t * P:(t + 1) * P], sct[:], ident[:])

                # chunk softmax: m_c, p_c, l_c
                nc.vector.reduce_max(m_all[:, c:c + 1], scT[:], axis=AX.X)
                nc.vector.tensor_scalar_mul(negm_all[:, c:c + 1], m_all[:, c:c + 1], -1.0)
                p_sb = p_pool.tile([B, CHUNK_TILES * P], F32)
                nc.scalar.activation(p_sb[:], scT[:], ACTF.Exp,
                                     bias=negm_all[:, c:c + 1], scale=1.0,
                                     accum_out=l_all[:, c:c + 1])

                # context partial: ctx[b, :] += sum_s p[s, b] * enc[s, b, :]
                # one matmul per (tile, b): prob column as 1-col fp32r weight,
                # f32r enc tile as moving data (1 cycle/row at N=512).
                for t in range(CHUNK_TILES):
                    j = c * CHUNK_TILES + t
                    ptp = psum_tr.tile([P, B], F32, tag="tr")
                    nc.tensor.transpose(ptp[:], p_sb[:, t * P:(t + 1) * P], ident[0:B, 0:B])
                    pz, pzd = pzs[j % NPZ], pz_diags[j % NPZ]
                    nc.scalar.copy(pzd, ptp[:])
                    for bb in range(B):
                        last = (t == CHUNK_TILES - 1 and bb == B - 1)
                        nc.tensor.matmul(
                            ctx_ps[:],
                            pz[:, bb, :],
                            enc_tiles[t][:, bb, :],
                            start=False, stop=last)
                nc.scalar.copy(ctxc_all[:, c, :], ctx_ps[:])

            # ---- combine chunks ----
            g_max = persist_pool.tile([B, 1], F32)
            g_negmax = persist_pool.tile([B, 1], F32)
            g_l = persist_pool.tile([B, 1], F32)
            g_rl = persist_pool.tile([B, 1], F32)
            nc.vector.reduce_max(g_max[:], m_all[:], axis=AX.X)
            nc.vector.tensor_scalar_mul(g_negmax[:], g_max[:], -1.0)
            nc.scalar.activation(w_all[:], m_all[:], ACTF.Exp,
                                 bias=g_negmax[:], scale=1.0)
            junk2 = persist_pool.tile([B, nchunk], F32)
            nc.vector.scalar_tensor_tensor(
                out=junk2[:], in0=l_all[:], scalar=1.0, in1=w_all[:],
                op0=ALU.mult, op1=ALU.mult, accum_out=g_l[:])
            nc.vector.reciprocal(g_rl[:], g_l[:])

            # normalized chunk weights: wn[b, c] = w[b, c] / l_total[b]
            w_norm = persist_pool.tile([B, nchunk], F32)
            nc.vector.tensor_scalar(out=w_norm[:], in0=w_all[:],
                                    scalar1=g_rl[:, 0:1], scalar2=None, op0=ALU.mult)

            # weighted sum over chunks, all in natural [b, d] layout
            acc = [persist_pool.tile([B, DE], F32, name=f"acc{i}")
                   for i in range(2)]
            nc.vector.tensor_scalar(out=acc[0][:], in0=ctxc_all[:, 0, :],
                                    scalar1=w_norm[:, 0:1], scalar2=None,
                                    op0=ALU.mult)
            for c in range(1, nchunk):
                src, dst = acc[(c - 1) % 2], acc[c % 2]
                nc.vector.scalar_tensor_tensor(
                    out=dst[:], in0=ctxc_all[:, c, :], scalar=w_norm[:, c:c + 1],
                    in1=src[:], op0=ALU.mult, op1=ALU.add)
            nc.sync.dma_start(out_d[:], acc[(nchunk - 1) % 2][:])
            _ctx_cm.__exit__(None, None, None)
            _sc_cm.__exit__(None, None, None)
            _tr_cm.__exit__(None, None, None)

    nc.compile()
    if not nc.is_finalized():
        nc.finalize()
    return nc


_NC = None


def kernel(decoder_hidden, encoder_output, W, b):
    global _NC
    if _NC is None:
        _NC = build_nc()
    decoder_hidden = np.ascontiguousarray(decoder_hidden, dtype=np.float32)
    encoder_output = np.ascontiguousarray(encoder_output, dtype=np.float32)
    W = np.ascontiguousarray(W, dtype=np.float32)
    b = np.ascontiguousarray(b, dtype=np.float32)

    in_maps = []
    for i in range(NCORES):
        sl = slice(i * B, (i + 1) * B)
        in_maps.append({
            "decoder_hidden": decoder_hidden[sl],
            "encoder_output": np.ascontiguousarray(encoder_output[:, sl, :]),
            "W": W,
            "b": b,
        })
    res = run_bass_kernel_spmd(_NC, in_maps, core_ids=list(range(NCORES)))
    return np.concatenate([res.results[i]["out"] for i in range(NCORES)], axis=0)


# revision 10
# speedup vs baseline: 1.0427x; 1.0427x over previous
"""Trainium2 Bass kernel for Luong-style attention.

Reference computation (per full problem):
    h = decoder_hidden @ W.T + b          # [B, De]
    enc = encoder_output.transpose(1,0,2) # [B, S, De]
    a = softmax(einsum('bsd,bd->bs', enc, h), axis=1)
    context = einsum('bs,bsd->bd', a, enc)  # [B, De]

Shapes: B=64, S=4096, Dd=1024, De=512 (f32).

Strategy: data-parallel over B across 8 NeuronCores (B_local=8 each).
encoder_output is the huge tensor (512 MB); each core streams its
64 MB shard from HBM exactly once (chunked two-level softmax).
Per 128-row s-tile (f32, no bf16 copy of the stream):
  - scores via DVE scalar_tensor_tensor (product + row-sum in one op)
    against a partition-broadcast fp16 copy of h (one 2-byte source
    keeps the DVE at full rate; two f32 sources would halve it),
  - per-chunk softmax via PE transpose + ACT exp (bias=-chunk_max,
    fused row-sum),
  - context accumulated in a single [8, 512] PSUM bank: for each b,
    matmul(lhsT=prob column [128,1] fp32r, rhs=enc f32r [128,512]) ->
    out row [1,512] at partition b.  fp32r moving data runs at
    1 cycle/row for N>=256, and a 1-column weight load is ~free, so
    the PE streams the f32 tile directly (no bf16 cast needed).
    The accumulation group is opened by one bank-wide zero matmul
    (start=True clears has_written bits bank-wide).
  - chunks combined at the end with exp(m_c - M)/l weights; partials
    are already in natural [b, d] layout so the combine is a short
    chain of [8,512] DVE ops.
No collectives needed.  Bottleneck is the HBM stream (~187 us for
64 MB/core at ~358 GB/s); all engines sit below that.
"""

import numpy as np

import concourse.bass as bass
import concourse.bacc as bacc_mod
import concourse.tile as tile
import concourse.mybir as mybir
from concourse import masks
from concourse.bass_utils import run_bass_kernel_spmd

F32 = mybir.dt.float32
F32R = mybir.dt.float32r
F16 = mybir.dt.float16
BF16 = mybir.dt.bfloat16
ALU = mybir.AluOpType
ACTF = mybir.ActivationFunctionType
AX = mybir.AxisListType

NCORES = 8
B = 8          # per-core batch
S = 4096
DD = 1024
DE = 512
P = 128        # s-values per tile
NTILES = S // P          # 32
CHUNK_TILES = 4          # s-tiles per softmax chunk
NCHUNK = NTILES // CHUNK_TILES   # 8


def build_nc(ntiles: int = NTILES):
    nchunk = ntiles // CHUNK_TILES
    s_local = ntiles * P
    nc = bacc_mod.Bacc("TRN2", target_bir_lowering=False, debug=False)
    dec_d = nc.dram_tensor("decoder_hidden", [B, DD], F32, kind="ExternalInput")
    enc_d = nc.dram_tensor("encoder_output", [s_local, B, DE], F32, kind="ExternalInput")
    w_d = nc.dram_tensor("W", [DE, DD], F32, kind="ExternalInput")
    b_d = nc.dram_tensor("b", [DE], F32, kind="ExternalInput")
    out_d = nc.dram_tensor("out", [B, DE], F32, kind="ExternalOutput")

    with tile.TileContext(nc) as tc:
        with (
            tc.tile_pool(name="const", bufs=1) as const_pool,
            tc.tile_pool(name="persist", bufs=1) as persist_pool,
            tc.tile_pool(name="enc", bufs=5) as enc_pool,
            tc.tile_pool(name="junk", bufs=6) as junk_pool,
            tc.tile_pool(name="scores", bufs=6) as sc_pool,
            tc.tile_pool(name="probs", bufs=2) as p_pool,
        ):
            wload_cm = tc.tile_pool(name="wload", bufs=2)
            wload_pool = wload_cm.__enter__()
            wt_cm = tc.tile_pool(name="wt", bufs=1)
            wt_pool = wt_cm.__enter__()
            setup_psum_cm = tc.tile_pool(name="psum_setup", bufs=4, space="PSUM")
            psum_setup = setup_psum_cm.__enter__()
            setup_psum2_cm = tc.tile_pool(name="psum_setup2", bufs=4, space="PSUM")
            psum_setup2 = setup_psum2_cm.__enter__()
            # ---- constants ----
            ident = const_pool.tile([P, P], F32)
            masks.make_identity(nc, ident[:])
            ones = const_pool.tile([1, P], F32)
            nc.vector.memset(ones[:], 1.0)
            zeros_row = const_pool.tile([1, DE], BF16)
            nc.vector.memset(zeros_row[:], 0.0)
            ones_bf = const_pool.tile([1, B], BF16)
            nc.vector.memset(ones_bf[:], 1.0)
            # row-broadcast selectors: sel[:, bb, :] is [8, 128] with row bb
            # all-ones; matmul(sel_bb, x) broadcasts x's row bb to all
            # 128 partitions without any cross-partition DMA.
            sel = const_pool.tile([B, B, P], F32)
            nc.gpsimd.memset(sel[:], 0.0)
            # sel[k, bb, m] = 1.0 iff k == bb  (k*1 + bb*(-1) == 0)
            nc.gpsimd.affine_select(
                out=sel[:], in_=sel[:],
                compare_op=ALU.not_equal, fill=1.0, base=0,
                pattern=[[-1, B], [0, P]], channel_multiplier=1)

            # ---- load small inputs ----
            dec_sb = const_pool.tile([B, DD], F32)
            nc.sync.dma_start(dec_sb[:], dec_d[:])
            bias_sb = const_pool.tile([1, DE], F32)
            nc.sync.dma_start(bias_sb[:], b_d[None, :])
            setup_dmas = []

            # ---- transpose dec: [8,1024] -> decT [128, 8, 8] (chunk c = cols c*128..) ----
            decT = const_pool.tile([P, DD // P, B], F32)
            for c in range(DD // P):
                tp = psum_setup.tile([P, B], F32, tag="su")
                nc.tensor.transpose(tp[:], dec_sb[:, c * P:(c + 1) * P], ident[0:B, 0:B])
                nc.vector.tensor_copy(decT[:, c, :], tp[:])

            # ---- transpose W: [512,1024] -> WT [128, 8, 512] (chunk c = W.T rows c*128..) ----
            wt_sb = wt_pool.tile([P, DD // P, DE], F32)
            for wi in range(DE // P):
                w_row = wload_pool.tile([P, DD], F32, tag="wrow")
                half = DD // 2
                setup_dmas.append(nc.sync.dma_start(
                    w_row[:, 0:half], w_d[wi * P:(wi + 1) * P, 0:half]))
                setup_dmas.append(nc.sync.dma_start(
                    w_row[:, half:DD], w_d[wi * P:(wi + 1) * P, half:DD]))
                for c in range(DD // P):
                    tp = psum_setup.tile([P, P], F32, tag="su")
                    nc.tensor.transpose(tp[:], w_row[:, c * P:(c + 1) * P], ident[:])
                    nc.vector.tensor_copy(wt_sb[:, c, wi * P:(wi + 1) * P], tp[:])

            # ---- h = dec @ W.T + b  -> h_sb [8, 512] ----
            h_ps = psum_setup2.tile([B, DE], F32, tag="hsu")
            for c in range(DD // P):
                nc.tensor.matmul(h_ps[:], decT[:, c, :], wt_sb[:, c, :],
                                 start=(c == 0), stop=False)
            nc.tensor.matmul(h_ps[:], ones[0:1, 0:B], bias_sb[:],
                             start=False, stop=True)
            h_sb = const_pool.tile([B, DE], F32)
            nc.vector.tensor_copy(h_sb[:], h_ps[:])

            # ---- broadcast h along partitions: hb [128, 8, 512] fp16 ----
            # selector matmul: out = sel_bb.T @ h_sb puts h row bb on all
            # 128 partitions; the psum->sbuf copy converts to fp16 so the
            # score stt has only one 4-byte source (full DVE rate).
            hb = persist_pool.tile([P, B, DE], F16)
            for bb in range(B):
                hp = psum_setup2.tile([P, DE], F32, tag="hsu")
                nc.tensor.matmul(hp[:], sel[:, bb, :], h_sb[:],
                                 start=True, stop=True)
                nc.scalar.copy(hb[:, bb, :], hp[:])

            setup_psum2_cm.__exit__(None, None, None)
            setup_psum_cm.__exit__(None, None, None)
            wt_cm.__exit__(None, None, None)
            wload_cm.__exit__(None, None, None)
            _tr_cm = tc.tile_pool(name="psum_tr", bufs=2, space="PSUM")
            psum_tr = _tr_cm.__enter__()
            _sc_cm = tc.tile_pool(name="psum_sc", bufs=2, space="PSUM")
            psum_sc = _sc_cm.__enter__()
            _ctx_cm = tc.tile_pool(name="psum_ctx", bufs=2, space="PSUM")
            psum_ctx = _ctx_cm.__enter__()

            # ---- diagonal prob-weight tiles ----
            # pz[p, b, b'] = p_col_b[p] iff b == b' else 0.  Used as the
            # [128, 8] lhsT for the context matmul of batch b: the single
            # nonzero column routes the output to psum partition b while
            # keeping the matmul's base partition at 0 (PE requires 0/32/64).
            # Off-diagonals are zeroed once here and never written again;
            # each tile only refreshes the 8 diagonal slots (stride-9 AP).
            NPZ = 4
            pzs = [persist_pool.tile([P, B, B], BF16, name=f"pz{i}")
                   for i in range(NPZ)]
            pz_diags = []
            for pz in pzs:
                nc.vector.memset(pz[:], 0.0)
                pz_diags.append(pz[:].rearrange("p a b -> p (a b)")[:, 0:B * B:B + 1])

            # ---- per-chunk stats / outputs ----
            m_all = persist_pool.tile([B, nchunk], F32)
            negm_all = persist_pool.tile([B, nchunk], F32)
            l_all = persist_pool.tile([B, nchunk], F32)
            w_all = persist_pool.tile([B, nchunk], F32)
            # context partials per chunk, natural [b, d] layout
            ctxc_all = persist_pool.tile([B, nchunk, DE], F32)

            # ---- main streaming loop over S ----
            for c in range(nchunk):
                # open the ctx accumulation bank early (no deps beyond pool
                # rotation) so the PE's in-order queue never stalls on it.
                ctx_ps = psum_ctx.tile([B, DE], F32)
                nc.tensor.matmul(ctx_ps[:], ones_bf[:], zeros_row[:],
                                 start=True, stop=False)
                prod_tiles = []
                scT = psum_sc.tile([B, CHUNK_TILES * P], F32)
                for t in range(CHUNK_TILES):
                    j = c * CHUNK_TILES + t
                    et = enc_pool.tile([P, B, DE], F32)
                    enc_dma = nc.sync.dma_start(et[:], enc_d[j * P:(j + 1) * P, :, :])
                    if j == 0:
                        for sd in setup_dmas:
                            tile.add_dep_helper(enc_dma.ins, sd.ins,
                                                reason="let setup W loads win HBM first")
                    # products enc*h_q in bf16; the row-sum accumulator
                    # yields the scores, and the product tile itself is the
                    # context matmul's moving operand (ctx = sum p*prod / h_q).
                    jt = junk_pool.tile([P, B, DE], BF16, tag="junk")
                    prod_tiles.append(jt)
                    sct = sc_pool.tile([P, B], F32)
                    for bb in range(B):
                        nc.vector.scalar_tensor_tensor(
                            out=jt[:, bb, :],
                            in0=et[:, bb, :],
                            scalar=1.0,
                            in1=hb[:, bb, :],
                            op0=ALU.mult,
                            op1=ALU.mult,
                            accum_out=sct[:, bb:bb + 1],
                        )
                    # transpose scores into [8, 128] slice of chunk psum
                    nc.tensor.transpose(scT[:, t * P:(t + 1) * P], sct[:], ident[:])

                # chunk softmax: m_c, p_c, l_c
                nc.vector.reduce_max(m_all[:, c:c + 1], scT[:], axis=AX.X)
                nc.vector.tensor_scalar_mul(negm_all[:, c:c + 1], m_all[:, c:c + 1], -1.0)
                p_sb = p_pool.tile([B, CHUNK_TILES * P], F32)
                nc.scalar.activation(p_sb[:], scT[:], ACTF.Exp,
                                     bias=negm_all[:, c:c + 1], scale=1.0,
                                     accum_out=l_all[:, c:c + 1])

                # context partial: ctx[b, :] += sum_s p[s, b] * enc[s, b, :]
                # one matmul per (tile, b): prob column as 1-col fp32r weight,
                # f32r enc tile as moving data (1 cycle/row at N=512).
                for t in range(CHUNK_TILES):
                    j = c * CHUNK_TILES + t
                    ptp = psum_tr.tile([P, B], F32, tag="tr")
                    nc.tensor.transpose(ptp[:], p_sb[:, t * P:(t + 1) * P], ident[0:B, 0:B])
                    pz, pzd = pzs[j % NPZ], pz_diags[j % NPZ]
                    nc.scalar.copy(pzd, ptp[:])
                    for bb in range(B):
                        last = (t == CHUNK_TILES - 1 and bb == B - 1)
                        nc.tensor.matmul(
                            ctx_ps[:],
                            pz[:, bb, :],
                            prod_tiles[t][:, bb, :],
                            start=False, stop=last)
                nc.scalar.copy(ctxc_all[:, c, :], ctx_ps[:])

            # ---- combine chunks ----
            g_max = persist_pool.tile([B, 1], F32)
            g_negmax = persist_pool.tile([B, 1], F32)
            g_l = persist_pool.tile([B, 1], F32)
            g_rl = persist_pool.tile([B, 1], F32)
            nc.vector.reduce_max(g_max[:], m_all[:], axis=AX.X)
            nc.vector.tensor_scalar_mul(g_negmax[:], g_max[:], -1.0)
            nc.scalar.activation(w_all[:], m_all[:], ACTF.Exp,
                                 bias=g_negmax[:], scale=1.0)
            junk2 = persist_pool.tile([B, nchunk], F32)
            nc.vector.scalar_tensor_tensor(
                out=junk2[:], in0=l_all[:], scalar=1.0, in1=w_all[:],
                op0=ALU.mult, op1=ALU.mult, accum_out=g_l[:])
            nc.vector.reciprocal(g_rl[:], g_l[:])

            # normalized chunk weights: wn[b, c] = w[b, c] / l_total[b]
            w_norm = persist_pool.tile([B, nchunk], F32)
            nc.vector.tensor_scalar(out=w_norm[:], in0=w_all[:],
                                    scalar1=g_rl[:, 0:1], scalar2=None, op0=ALU.mult)

            # weighted sum over chunks, all in natural [b, d] layout
            acc = [persist_pool.tile([B, DE], F32, name=f"acc{i}")
                   for i in range(2)]
            nc.vector.tensor_scalar(out=acc[0][:], in0=ctxc_all[:, 0, :],
                                    scalar1=w_norm[:, 0:1], scalar2=None,
                                    op0=ALU.mult)
            for c in range(1, nchunk):
                src, dst = acc[(c - 1) % 2], acc[c % 2]
                nc.vector.scalar_tensor_tensor(
                    out=dst[:], in0=ctxc_all[:, c, :], scalar=w_norm[:, c:c + 1],
                    in1=src[:], op0=ALU.mult, op1=ALU.add)
            # ctx partials carry a factor h_q (products were enc*h_q);
            # divide it out with the reciprocal of the same fp16 rounding.
            hq = persist_pool.tile([B, DE], F16)
            nc.scalar.copy(hq[:], h_sb[:])
            h_rcp = persist_pool.tile([B, DE], F32)
            nc.vector.reciprocal(h_rcp[:], hq[:])
            final_sb = persist_pool.tile([B, DE], F32)
            nc.vector.tensor_tensor(out=final_sb[:], in0=acc[(nchunk - 1) % 2][:],
                                    in1=h_rcp[:], op=ALU.mult)
            nc.sync.dma_start(out_d[:], final_sb[:])
            _ctx_cm.__exit__(None, None, None)
            _sc_cm.__exit__(None, None, None)
            _tr_cm.__exit__(None, None, None)

    nc.compile()
    if not nc.is_finalized():
        nc.finalize()
    return nc


_NC = None


def kernel(decoder_hidden, encoder_output, W, b):
    global _NC
    if _NC is None:
        _NC = build_nc()
    decoder_hidden = np.ascontiguousarray(decoder_hidden, dtype=np.float32)
    encoder_output = np.ascontiguousarray(encoder_output, dtype=np.float32)
    W = np.ascontiguousarray(W, dtype=np.float32)
    b = np.ascontiguousarray(b, dtype=np.float32)

    in_maps = []
    for i in range(NCORES):
        sl = slice(i * B, (i + 1) * B)
        in_maps.append({
            "decoder_hidden": decoder_hidden[sl],
            "encoder_output": np.ascontiguousarray(encoder_output[:, sl, :]),
            "W": W,
            "b": b,
        })
    res = run_bass_kernel_spmd(_NC, in_maps, core_ids=list(range(NCORES)))
    return np.concatenate([res.results[i]["out"] for i in range(NCORES)], axis=0)
